# revision 9
# baseline (speedup 1.0000x reference)
"""GAT (graph attention) layer on 8 TRN2 NeuronCores — Bass/Tile kernel.

Sharding: destination-node dim i is split across the 8 cores (256 rows
each).  Wh and params are replicated; softmax is over j within a row so
no collective is needed.

Math rewrite (per core, rows i in its shard):
  e = leaky_relu(e_i + e_j);  exp(lrelu(s)) = max(exp(s), exp(alpha*s))
  Dividing the softmax weights by exp(e_i)*exp(e_j) (cancels in the
  normalization):
    weight[i,j,h]  propto  v_j * min(max(r_i * w_j, 1), HUGE*adj)
  with v_j = exp(e_j), w_j = exp((alpha-1) e_j), r_i = exp((alpha-1) e_i).
  So NO transcendentals on the [N,NL,H] logits tensor: exps only on the
  small [N,H] vectors.  Per j-tile:
    PE:  Y2[j,(h,i)] = r_i * w_j   (rank-8 matmul, K=8)
    ACT: T0 = bf16(Y2)             (PSUM evacuation)
    DVE: G = (T0 max 1.0) min adjHUGE   (one fused scalar_tensor_tensor)
    PE:  agg[i,(h,d)] += G^T @ (v_j*Wh_aug[j,h,:])  per head
  v folds into the agg rhs (VWh); its ones-column gives the denominator.

Host-side prep: hT pre-transposed into [nt,k,128,128] bf16 blocks,
W_aug = [W | W@a_j_h | W@a_i_h] folded in numpy, adjT scaled by 1e30.
"""

import dataclasses
import sys

import numpy as np

sys.path.insert(0, "/opt/trn_rl_repo")

N = 2048
F_IN = 768
F_OUT = 64
H = 8
ALPHA = 0.2
NCORES = 8
NL = N // NCORES          # 256 local rows per core
KT = F_IN // 128          # 6 k-tiles
NT = N // 128             # 16 j tiles
FH = F_OUT * H            # 512
FW = FH + 2 * H           # 528: [W | wa_j | wa_i] folded rhs
HUGE = 1e30

_CACHE = {}


def _build():
    import concourse.bacc as bacc
    import concourse.mybir as mybir
    from concourse.tile import TileContext

    f32 = mybir.dt.float32
    bf16 = mybir.dt.bfloat16
    AF = mybir.ActivationFunctionType
    OP = mybir.AluOpType

    nc = bacc.Bacc("TRN2", target_bir_lowering=False, debug=False,
                   num_devices=NCORES)

    # hT blocks: [nt, k, 128, 128] flattened; block (t,k) is h[t*128:(t+1)*128,
    # k*128:(k+1)*128].T contiguous.
    hT_d = nc.declare_dram_parameter("hT", [NT * KT * 128, 128], bf16,
                                     isOutput=False)
    hlT_d = nc.declare_dram_parameter("hlT", [KT * 128, NL], bf16,
                                      isOutput=False)
    Waug_d = nc.declare_dram_parameter("Waug", [KT * 128, FW], bf16,
                                       isOutput=False)
    adjH_d = nc.declare_dram_parameter("adjH", [N, NL], bf16, isOutput=False)
    out_d = nc.declare_dram_parameter("out", [NL, FH], f32, isOutput=True)

    with TileContext(nc) as tc:
        with tc.tile_pool(name="persist", bufs=1) as pp:
            ident = pp.tile([128, 128], f32)
            Waug_sb = pp.tile([128, KT, FW], bf16)
            hlT_sb = pp.tile([128, KT, 2, 128], bf16)
            adjH_sb = pp.tile([128, NT, NL], bf16)
            rhsY2 = pp.tile([H, H * NL], bf16)
            r_sb = pp.tile([128, NT], f32)
            dn_sb = pp.tile([128, NT], f32)
            hp_sb = pp.tile([128, 2, FH], f32)
            mn_sb = pp.tile([128, 2, FH], f32)
            em_sb = pp.tile([128, 2, FH], f32)
            out_sb = pp.tile([128, 2, FH], f32)
            zs_row = pp.tile([1, 512], f32)

            # ---------- pre-phase: params, e_i -> rhsY2 ----------
            with tc.tile_pool(name="pre", bufs=2) as sp, \
                 tc.tile_pool(name="preps", bufs=2, space="PSUM") as ps:

                io_t = sp.tile([128, 128], mybir.dt.int32, tag="iota")
                nc.gpsimd.iota(io_t[:], pattern=[[-1, 128]], base=0,
                               channel_multiplier=1)
                nc.vector.tensor_scalar(ident[:], io_t[:], 0, None,
                                        OP.is_equal)
                nc.vector.memset(zs_row[:], 0.0)
                nc.vector.memset(rhsY2[:], 0.0)

                for k in range(KT):
                    nc.sync.dma_start(out=Waug_sb[:, k, :],
                                      in_=Waug_d[k * 128:(k + 1) * 128, :])
                for k in range(KT):
                    nc.sync.dma_start(
                        out=hlT_sb[:, k, :, :].rearrange("p a b -> p (a b)"),
                        in_=hlT_d[k * 128:(k + 1) * 128, :])
                for jt in range(NT):
                    nc.sync.dma_start(out=adjH_sb[:, jt, :],
                                      in_=adjH_d[jt * 128:(jt + 1) * 128, :])

                # e_i per local half-tile; r_i = exp((a-1)e_i) into rhsY2
                # diagonal blocks (rhsY2[h, h*NL+lt*128 : ...]).
                for lt in range(2):
                    ps_ei = ps.tile([128, H], f32, tag="ei")
                    for k in range(KT):
                        nc.tensor.matmul(ps_ei[:], hlT_sb[:, k, lt, :],
                                         Waug_sb[:, k, FH + H:FW],
                                         start=(k == 0), stop=(k == KT - 1))
                    r32 = sp.tile([128, H], f32, tag="r32")
                    nc.scalar.activation(r32[:], ps_ei[:], AF.Exp,
                                         scale=ALPHA - 1.0)
                    ps_rT = ps.tile([H, 128], f32, tag="rT")
                    nc.tensor.transpose(ps_rT[:], r32[:], ident[:])
                    rT = sp.tile([H, 128], bf16, tag="rTb")
                    nc.vector.tensor_copy(rT[:], ps_rT[:])
                    for hh in range(H):
                        nc.sync.dma_start(
                            out=rhsY2[hh:hh + 1,
                                      hh * NL + lt * 128:
                                      hh * NL + (lt + 1) * 128],
                            in_=rT[hh:hh + 1, :])

            # ---------- main loop over j-tiles ----------
            with tc.tile_pool(name="ht", bufs=3) as hp_pool, \
                 tc.tile_pool(name="ev", bufs=2) as ev, \
                 tc.tile_pool(name="gv", bufs=2) as gv, \
                 tc.tile_pool(name="psm", bufs=1, space="PSUM") as psm, \
                 tc.tile_pool(name="psy", bufs=1, space="PSUM") as pyp:

                ps_agg = psm.tile([128, NT, F_OUT + 1], f32, tag="agg")

                # zero the agg banks once (16 groups share banks; start=True
                # would wipe neighbors), then accumulate with start=False.
                agg_flat = ps_agg[:].rearrange("p g d -> p (g d)")
                tot = NT * (F_OUT + 1)
                off = 0
                while off < tot:
                    w = min(512, tot - off)
                    nc.tensor.matmul(agg_flat[:, off:off + w],
                                     zs_row[0:1, 0:128],
                                     zs_row[0:1, 0:w],
                                     start=True, stop=False,
                                     skip_group_check=True)
                    off += w

                HW2 = H // 2 * NL            # 1024 cols per half
                prev = None                   # (G, VWh) of previous tile

                def agg_mm(Gt, VWht, jt):
                    for hh in range(H):
                        for ih in range(2):
                            g = hh * 2 + ih
                            nc.tensor.matmul(
                                ps_agg[:, g, :],
                                Gt[:, hh * NL + ih * 128:
                                   hh * NL + ih * 128 + 128],
                                VWht[:, hh, :],
                                start=False, stop=(jt == NT - 1),
                                skip_group_check=True)

                for jt in range(NT):
                    hT_t = hp_pool.tile([128, KT, 128], bf16, tag="ht")
                    for k in range(KT):
                        nc.sync.dma_start(
                            out=hT_t[:, k, :],
                            in_=hT_d[(jt * KT + k) * 128:
                                     (jt * KT + k + 1) * 128, :])

                    # Wh + e_j cols
                    # matmul PSUM out must stay within one bank: split
                    # the 520 cols into 512 (bank 0) + 8 (bank 1).
                    ps_w = psm.tile([128, FH + H], f32, tag="w")
                    for k in range(KT):
                        nc.tensor.matmul(ps_w[:, 0:FH], hT_t[:, k, :],
                                         Waug_sb[:, k, 0:FH],
                                         start=(k == 0), stop=(k == KT - 1))
                        nc.tensor.matmul(ps_w[:, FH:FH + H], hT_t[:, k, :],
                                         Waug_sb[:, k, FH:FH + H],
                                         start=(k == 0), stop=(k == KT - 1))
                    v_sb = ev.tile([128, H], bf16, tag="v")
                    nc.scalar.activation(v_sb[:], ps_w[:, FH:FH + H], AF.Exp)
                    w32 = ev.tile([128, H], f32, tag="w32")
                    nc.scalar.activation(w32[:], ps_w[:, FH:FH + H], AF.Exp,
                                         scale=ALPHA - 1.0)
                    ps_wT = psm.tile([H, 128], f32, tag="wt")
                    nc.tensor.transpose(ps_wT[:], w32[:], ident[:])

                    wT = ev.tile([H, 128], bf16, tag="wT")
                    nc.vector.tensor_copy(wT[:], ps_wT[:])

                    # VWh = v * Wh (+ v into the ones column).  Iterate
                    # d-outer / h-inner so the broadcast v operand keeps a
                    # stride-1 innermost dim (walrus rejects stride-0 there).
                    VWh = gv.tile([128, H, F_OUT + 1], bf16, tag="vwh")
                    out_ap = dataclasses.replace(
                        VWh[:], ap=[list(VWh[:].ap[0]),
                                    [1, F_OUT], [F_OUT + 1, H]])
                    in0_ap = dataclasses.replace(
                        ps_w[:], ap=[list(ps_w[:].ap[0]),
                                     [1, F_OUT], [F_OUT, H]])
                    v_rep = dataclasses.replace(
                        v_sb[:], ap=[list(v_sb[:].ap[0]),
                                     [0, F_OUT], [1, H]])
                    nc.vector.tensor_tensor(out_ap, in0_ap, v_rep, OP.mult)
                    nc.vector.tensor_copy(VWh[:, :, F_OUT], v_sb[:])

                    # Y2 = r_i (x) w_j per head, in halves; evac via ACT;
                    # fused mask+max on DVE.  psy is single-buffered: the
                    # lagged agg matmuls run between the halves to cover the
                    # WAR wait on the ACT evacuation.
                    G = gv.tile([128, H * NL], bf16, tag="g")
                    base = adjH_sb[:, jt, :]
                    for hf in range(2):
                        f0 = hf * HW2
                        ps_y = pyp.tile([128, HW2], f32, tag="y")
                        for q in range(2):
                            nc.tensor.matmul(
                                ps_y[:, q * 512:(q + 1) * 512], wT[:],
                                rhsY2[:, f0 + q * 512:f0 + (q + 1) * 512],
                                start=True, stop=True)
                        if hf == 0 and prev is not None:
                            agg_mm(*prev, jt - 1)
                        T0 = ev.tile([128, HW2], bf16, tag="t0")
                        nc.scalar.copy(out=T0[:], in_=ps_y[:])
                        rep = dataclasses.replace(
                            base, ap=[list(base.ap[0]), [0, H // 2],
                                      list(base.ap[1])])
                        nc.vector.scalar_tensor_tensor(
                            G[:, f0:f0 + HW2].rearrange(
                                "p (h i) -> p h i", h=H // 2),
                            T0[:].rearrange("p (h i) -> p h i", h=H // 2),
                            1.0, rep, OP.max, OP.min)

                    prev = (G, VWh)

                agg_mm(*prev, NT - 1)

                # ---------- finalize: normalize + ELU + store ----------
                nc.vector.tensor_copy(dn_sb[:], ps_agg[:, :, F_OUT])
                nc.vector.reciprocal(r_sb[:], dn_sb[:])
                for hh in range(H):
                    for ih in range(2):
                        g = hh * 2 + ih
                        nc.vector.tensor_scalar(
                            hp_sb[:, ih, hh * F_OUT:(hh + 1) * F_OUT],
                            ps_agg[:, g, 0:F_OUT],
                            r_sb[:, g:g + 1], None, OP.mult)
                nc.vector.tensor_scalar(mn_sb[:], hp_sb[:], 0.0, None, OP.min)
                nc.scalar.activation(em_sb[:], mn_sb[:], AF.Exp)
                nc.vector.scalar_tensor_tensor(out_sb[:], em_sb[:], -1.0,
                                               hp_sb[:], OP.add, OP.max)
                for ih in range(2):
                    nc.sync.dma_start(out=out_d[ih * 128:(ih + 1) * 128, :],
                                      in_=out_sb[:, ih, :])

    nc.compile()
    return nc


def kernel(h, adj, W, a):
    import ml_dtypes
    from concourse.bass_utils import run_bass_kernel_spmd

    if "nc" not in _CACHE:
        _CACHE["nc"] = _build()
    nc = _CACHE["nc"]

    bf16 = ml_dtypes.bfloat16
    h = np.ascontiguousarray(h, dtype=np.float32)
    adj = np.ascontiguousarray(adj, dtype=np.float32)
    W = np.ascontiguousarray(W, dtype=np.float32)
    a = np.ascontiguousarray(a, dtype=np.float32)

    a_i = a[0, :, :F_OUT]                    # [H, D]
    a_j = a[0, :, F_OUT:]                    # [H, D]
    Wr = W.reshape(F_IN, H, F_OUT)
    Wa_j = np.einsum('khd,hd->kh', Wr, a_j)  # [F_IN, H]
    Wa_i = np.einsum('khd,hd->kh', Wr, a_i)
    Waug = np.concatenate([W, Wa_j, Wa_i], axis=1).astype(bf16)  # [768, 528]

    # hT blocks [nt, k, 128, 128]: block = h[t-tile, k-tile].T
    hT = np.ascontiguousarray(
        h.reshape(NT, 128, KT, 128).transpose(0, 2, 3, 1)).astype(bf16)
    hT = hT.reshape(NT * KT * 128, 128)

    in_maps = []
    for c in range(NCORES):
        sl = slice(c * NL, (c + 1) * NL)
        hl = h[sl]                            # [256, 768]
        hlT = np.ascontiguousarray(
            hl.reshape(2, 128, KT, 128).transpose(2, 3, 0, 1)).astype(bf16)
        hlT = hlT.reshape(KT * 128, NL)       # [k*128, (lt,128)]
        adjH = ((adj[sl].T > 0) * np.float32(HUGE)).astype(bf16)
        in_maps.append({
            "hT": hT,
            "hlT": hlT,
            "Waug": Waug,
            "adjH": np.ascontiguousarray(adjH),
        })
    res = run_bass_kernel_spmd(nc, in_maps, list(range(NCORES)),
                               trace=bool(_CACHE.get("trace")))
    _CACHE["last"] = res
    return np.concatenate([res.results[c]["out"] for c in range(NCORES)],
                          axis=0)
